# revision 1
# baseline (speedup 1.0000x reference)
"""Trainium2 Bass kernel: LiquidCell (9-step Euler scan over a 3-layer MLP + LN).

Strategy: pure data parallel over 8 NeuronCores (B=131072 -> 16384 rows/core).
On-chip layout is feature-major (activations transposed: [feat_part, batch_free])
so every matmul chains without transposes; weights are pre-transposed on host.
bf16 matmuls with f32 PSUM accumulate; h kept in f32. LayerNorm mean is folded
into an extra matmul row (w_mu), sum-of-squares via ones-vector matmuls.
4 batch tiles are interleaved per hardware-loop iteration to hide the LN-stats
serial chain behind other tiles' matmul streams.
"""

import numpy as np

P = 128
NCORES = 8
BT = 512          # batch tile (matmul free dim)
G = 4             # tiles interleaved per loop body
LN_EPS = 1e-5


def _emit(nc, tc, d, BC, S, c1, has_b3, has_b1):
    import concourse.mybir as mybir
    from concourse.bass import ds, ts
    from contextlib import ExitStack

    f32 = mybir.dt.float32
    bf16 = mybir.dt.bfloat16
    AF = mybir.ActivationFunctionType
    OP = mybir.AluOpType
    ET = mybir.EngineType

    NS = S - 1            # Euler steps
    H = d["goutp"].shape[0] * d["goutp"].shape[1]     # 256
    H4 = d["w1t"].shape[2]                             # 1024
    H2 = d["w2t"].shape[2]                             # 512
    n_k1 = d["w1t"].shape[1]   # 4   (KIN/128)
    n_m1 = H4 // P             # 8
    n_m2 = H2 // P             # 4
    n_c = H // P               # 2

    with ExitStack() as ctx:
        singles = ctx.enter_context(tc.tile_pool(name="singles", bufs=1))
        pool_io = ctx.enter_context(tc.tile_pool(name="io", bufs=G))
        pool_p = ctx.enter_context(tc.tile_pool(name="pp", bufs=G + 1))
        pool_big = ctx.enter_context(tc.tile_pool(name="big", bufs=G))
        pool_bc = ctx.enter_context(tc.tile_pool(name="bc", bufs=2 * G))
        pool_f32 = ctx.enter_context(tc.tile_pool(name="f32tmp", bufs=8))
        pool_rf = ctx.enter_context(tc.tile_pool(name="rowf", bufs=8))
        pool_rb = ctx.enter_context(tc.tile_pool(name="rowb", bufs=6))
        psum_mm = ctx.enter_context(tc.tile_pool(name="psmm", bufs=5, space="PSUM"))
        psum_st = ctx.enter_context(tc.tile_pool(name="psst", bufs=3, space="PSUM"))

        def load(name, dtype):
            t = singles.tile(list(d[name].shape), dtype, tag=name)
            nc.sync.dma_start(out=t, in_=d[name][:])
            return t

        w1t = load("w1t", bf16)
        w2t = load("w2t", bf16)
        w3t = load("w3t", bf16)
        wmu = load("wmu", bf16)
        b1p = load("b1p", f32)
        g1p = load("g1p", f32)
        be1p = load("be1p", f32)
        b2p = load("b2p", f32)
        tmdtp = load("tmdtp", f32)
        goutp = load("goutp", f32)
        beoutp = load("beoutp", f32)
        b3row = load("b3row", bf16) if has_b3 else None

        ones1_bf = singles.tile([1, P], bf16)
        nc.vector.memset(ones1_bf, 1.0)
        ones_ss = singles.tile([P, 1], bf16)
        nc.vector.memset(ones_ss, 1.0 / H4)
        ones_hf = singles.tile([P, 1], f32)
        nc.vector.memset(ones_hf, 1.0 / H)
        ones1_f = singles.tile([1, P], f32)
        nc.vector.memset(ones1_f, 1.0)
        eps_t = singles.tile([1, 1], f32)
        nc.vector.memset(eps_t, LN_EPS)
        if has_b3:
            onesrow_bf = singles.tile([1, BT], bf16)
            nc.vector.memset(onesrow_bf, 1.0)

        class T:
            pass

        def make_tile(off):
            t = T()
            t.off = off
            t.xbf = pool_io.tile([P, 2, BT], bf16, tag="xbf")
            t.hT = pool_io.tile([P, 2, BT], f32, tag="hT")
            t.hbf = pool_io.tile([P, 2, BT], bf16, tag="hbf")
            nc.sync.dma_start(out=t.xbf, in_=d["xbf"][:, :, ds(off, BT)])
            nc.sync.dma_start(out=t.hT, in_=d["hf"][:, :, ds(off, BT)])
            nc.sync.dma_start(out=t.hbf, in_=d["hbf"][:, :, ds(off, BT)])
            return t

        def rhs_k(t, k):
            # hx = [h(2 chunks); x(2 chunks)] feature-major, bf16
            if k < 2:
                return t.hbf[:, k, :]
            return t.xbf[:, k - 2, :]

        def p1(t):
            """wmu + z1 matmuls, psum->sbuf drain (+b1), squares."""
            t.mu_ps = psum_st.tile([1, BT], f32, tag="st")
            for k in range(n_k1):
                nc.tensor.matmul(t.mu_ps, lhsT=wmu[:, k:k + 1], rhs=rhs_k(t, k),
                                 start=(k == 0), stop=(k == n_k1 - 1))
            t.s = pool_big.tile([P, n_m1, BT], bf16, tag="s")
            t.q = pool_big.tile([P, n_m1, BT], bf16, tag="q")
            for m in range(n_m1):
                zps = psum_mm.tile([P, BT], f32, tag="mm")
                for k in range(n_k1):
                    nc.tensor.matmul(zps, lhsT=w1t[:, k, ts(m, P)], rhs=rhs_k(t, k),
                                     start=(k == 0), stop=(k == n_k1 - 1))
                if has_b1:
                    nc.scalar.activation(out=t.s[:, m, :], in_=zps,
                                         func=AF.Identity,
                                         bias=b1p[:, m:m + 1], scale=1.0)
                elif m % 2 == 0:
                    nc.vector.tensor_copy(out=t.s[:, m, :], in_=zps)
                else:
                    nc.scalar.copy(out=t.s[:, m, :], in_=zps)
            nc.vector.tensor_mul(out=t.q, in0=t.s, in1=t.s)

        def p2a(t):
            """sum-of-squares matmuls + stats chain -> mu_bf, rs_bf rows."""
            t.ss_ps = psum_st.tile([1, BT], f32, tag="st")
            for m in range(n_m1):
                nc.tensor.matmul(t.ss_ps, lhsT=ones_ss, rhs=t.q[:, m, :],
                                 start=(m == 0), stop=(m == n_m1 - 1))
            mu_f = pool_rf.tile([1, BT], f32, tag="rf")
            nc.scalar.add(out=mu_f, in_=t.mu_ps, add=c1)
            t.mu_bf = pool_rb.tile([1, BT], bf16, tag="rb")
            nc.vector.tensor_copy(out=t.mu_bf, in_=mu_f)
            msq = pool_rf.tile([1, BT], f32, tag="rf")
            nc.vector.tensor_mul(out=msq, in0=mu_f, in1=mu_f)
            var = pool_rf.tile([1, BT], f32, tag="rf")
            nc.vector.tensor_sub(out=var, in0=t.ss_ps, in1=msq)
            # rsqrt(var+eps) = exp(-0.5*ln(var+eps))  (ACT Rsqrt is banned)
            lnv = pool_rf.tile([1, BT], f32, tag="rf")
            nc.scalar.activation(out=lnv, in_=var, func=AF.Ln,
                                 bias=eps_t[0:1, 0:1], scale=1.0)
            rs_f = pool_rf.tile([1, BT], f32, tag="rf")
            nc.scalar.activation(out=rs_f, in_=lnv, func=AF.Exp,
                                 bias=0.0, scale=-0.5)
            t.rs_bf = pool_rb.tile([1, BT], bf16, tag="rb")
            nc.vector.tensor_copy(out=t.rs_bf, in_=rs_f)

        def p2c(t):
            """broadcast stats across partitions, normalize, gelu -> z1g in t.s"""
            bmu = psum_mm.tile([P, BT], f32, tag="mm")
            nc.tensor.matmul(bmu, lhsT=ones1_bf, rhs=t.mu_bf, start=True, stop=True)
            brs = psum_mm.tile([P, BT], f32, tag="mm")
            nc.tensor.matmul(brs, lhsT=ones1_bf, rhs=t.rs_bf, start=True, stop=True)
            muB = pool_bc.tile([P, BT], bf16, tag="bc")
            nc.vector.tensor_copy(out=muB, in_=bmu)
            rsB = pool_bc.tile([P, BT], bf16, tag="bc")
            nc.vector.tensor_copy(out=rsB, in_=brs)
            u = t.q  # reuse squares tile
            nc.vector.tensor_sub(out=u, in0=t.s,
                                 in1=muB[:, None, :].to_broadcast((P, n_m1, BT)))
            nc.vector.tensor_mul(out=u, in0=u,
                                 in1=rsB[:, None, :].to_broadcast((P, n_m1, BT)))
            for m in range(n_m1):
                nc.scalar.activation(out=t.s[:, m, :], in_=u[:, m, :], func=AF.Gelu,
                                     bias=be1p[:, m:m + 1], scale=g1p[:, m:m + 1])

        def p3(t):
            t.p = pool_p.tile([P, n_m2, BT], bf16, tag="p")
            for m in range(n_m2):
                zps = psum_mm.tile([P, BT], f32, tag="mm")
                for k in range(n_m1):
                    nc.tensor.matmul(zps, lhsT=w2t[:, k, ts(m, P)], rhs=t.s[:, k, :],
                                     start=(k == 0), stop=(k == n_m1 - 1))
                nc.scalar.activation(out=t.p[:, m, :], in_=zps, func=AF.Gelu,
                                     bias=b2p[:, m:m + 1], scale=1.0)

        def p4(t, s):
            for c in range(n_c):
                dps = psum_mm.tile([P, BT], f32, tag="mm")
                for k in range(n_m2):
                    nc.tensor.matmul(dps, lhsT=w3t[:, k, ts(c, P)], rhs=t.p[:, k, :],
                                     start=(k == 0),
                                     stop=(k == n_m2 - 1) and not has_b3)
                if has_b3:
                    nc.tensor.matmul(dps, lhsT=b3row[0:1, ts(c, P)], rhs=onesrow_bf,
                                     start=False, stop=True)
                # h += tanh(t*wt+bt)*dt * dh   (tmdt precomputed per step on host)
                nc.vector.scalar_tensor_tensor(
                    out=t.hT[:, c, :], in0=dps, scalar=tmdtp[:, c, s:s + 1],
                    in1=t.hT[:, c, :], op0=OP.mult, op1=OP.add)
                if s < NS - 1:
                    nc.scalar.copy(out=t.hbf[:, c, :], in_=t.hT[:, c, :])

        def pfinal(t):
            """Final LayerNorm in f32 (f32 matmuls for the reductions) + store."""
            mu_ps = psum_st.tile([1, BT], f32, tag="st")
            for c in range(n_c):
                nc.tensor.matmul(mu_ps, lhsT=ones_hf, rhs=t.hT[:, c, :],
                                 start=(c == 0), stop=(c == n_c - 1))
            q2a = pool_f32.tile([P, BT], f32, tag="f32")
            nc.vector.tensor_mul(out=q2a, in0=t.hT[:, 0, :], in1=t.hT[:, 0, :])
            q2b = pool_f32.tile([P, BT], f32, tag="f32")
            nc.vector.tensor_mul(out=q2b, in0=t.hT[:, 1, :], in1=t.hT[:, 1, :])
            tsum = pool_f32.tile([P, BT], f32, tag="f32")
            nc.vector.tensor_add(out=tsum, in0=q2a, in1=q2b)
            ss_ps = psum_st.tile([1, BT], f32, tag="st")
            nc.tensor.matmul(ss_ps, lhsT=ones_hf, rhs=tsum, start=True, stop=True)
            mu2 = pool_rf.tile([1, BT], f32, tag="rf")
            nc.vector.tensor_copy(out=mu2, in_=mu_ps)
            msq2 = pool_rf.tile([1, BT], f32, tag="rf")
            nc.vector.tensor_mul(out=msq2, in0=mu2, in1=mu2)
            var2 = pool_rf.tile([1, BT], f32, tag="rf")
            nc.vector.tensor_sub(out=var2, in0=ss_ps, in1=msq2)
            lnv2 = pool_rf.tile([1, BT], f32, tag="rf")
            nc.scalar.activation(out=lnv2, in_=var2, func=AF.Ln,
                                 bias=eps_t[0:1, 0:1], scale=1.0)
            rs2 = pool_rf.tile([1, BT], f32, tag="rf")
            nc.scalar.activation(out=rs2, in_=lnv2, func=AF.Exp,
                                 bias=0.0, scale=-0.5)
            bmu = psum_mm.tile([P, BT], f32, tag="mm")
            nc.tensor.matmul(bmu, lhsT=ones1_f, rhs=mu2, start=True, stop=True)
            brs = psum_mm.tile([P, BT], f32, tag="mm")
            nc.tensor.matmul(brs, lhsT=ones1_f, rhs=rs2, start=True, stop=True)
            for c in range(n_c):
                dd = pool_f32.tile([P, BT], f32, tag="f32")
                nc.vector.tensor_sub(out=dd, in0=t.hT[:, c, :], in1=bmu)
                nc.vector.tensor_mul(out=dd, in0=dd, in1=brs)
                oc = pool_f32.tile([P, BT], f32, tag="f32")
                nc.scalar.activation(out=oc, in_=dd, func=AF.Identity,
                                     bias=beoutp[:, c:c + 1], scale=goutp[:, c:c + 1])
                nc.sync.dma_start(out=d["out"][:, c, ds(t.off, BT)], in_=oc)

        assert BC % (G * BT) == 0
        with tc.For_i(0, BC, G * BT,
                      hint_engines=(ET.PE, ET.Activation, ET.DVE)) as i0:
            tiles = []
            for g in range(G):
                tiles.append(make_tile(i0 if g == 0 else i0 + g * BT))
            for s in range(NS):
                # software-pipelined emission: stats of tile j-1/j-2 hide
                # behind tile j's z1 matmul stream
                for j in range(G):
                    p1(tiles[j])
                    if j >= 1:
                        p2a(tiles[j - 1])
                    if j >= 2:
                        p2c(tiles[j - 2])
                p2a(tiles[G - 1])
                p2c(tiles[G - 2])
                p2c(tiles[G - 1])
                for j in range(G):
                    p3(tiles[j])
                    p4(tiles[j], s)
            for j in range(G):
                pfinal(tiles[j])


def build_nc(BC, S, c1, has_b3, shapes, has_b1=False):
    import concourse.bass as bass
    import concourse.mybir as mybir
    import concourse.tile as tile

    f32 = mybir.dt.float32
    bf16 = mybir.dt.bfloat16
    nc = bass.Bass("TRN2", target_bir_lowering=False, debug=False)
    d = {}
    dts = {"xbf": bf16, "hf": f32, "hbf": bf16, "w1t": bf16, "w2t": bf16,
           "w3t": bf16, "wmu": bf16, "b1p": f32, "g1p": f32, "be1p": f32,
           "b2p": f32, "tmdtp": f32, "goutp": f32, "beoutp": f32,
           "b3row": bf16}
    for name, shape in shapes.items():
        d[name] = nc.dram_tensor(name, list(shape), dts[name],
                                 kind="ExternalInput")[:]
    d["out"] = nc.dram_tensor("out", [P, 2, BC], f32, kind="ExternalOutput")[:]
    with tile.TileContext(nc) as tc:
        _emit(nc, tc, d, BC, S, c1, has_b3, has_b1)
    return nc


def _fold_w(W):
    """W [m, kin] -> lhsT tiles [128, kin/128, m] with [kp, k, j] = W[j, k*128+kp]."""
    m, kin = W.shape
    return np.ascontiguousarray(W.T.reshape(kin // P, P, m).transpose(1, 0, 2))


def _fold_v(v):
    """v [F] -> [128, F/128] with [p, c] = v[c*128+p]."""
    return np.ascontiguousarray(v.reshape(-1, P).T)


def _fold_bm(a):
    """a [N, F] batch-major -> feature-major [128, F/128, N]."""
    n, f = a.shape
    return np.ascontiguousarray(a.T.reshape(f // P, P, n).transpose(1, 0, 2))


def prepare(W1, b1, g1, be1, W2, b2, W3, b3, wt, bt, g_out, be_out, S):
    import ml_dtypes
    bf = ml_dtypes.bfloat16
    f32 = np.float32

    const = {}
    const["w1t"] = _fold_w(W1).astype(bf)
    const["w2t"] = _fold_w(W2).astype(bf)
    const["w3t"] = _fold_w(W3).astype(bf)
    const["wmu"] = _fold_v(W1.mean(axis=0).astype(f32)).astype(bf)
    const["b1p"] = _fold_v(b1.astype(f32))
    const["g1p"] = _fold_v(g1.astype(f32))
    const["be1p"] = _fold_v(be1.astype(f32))
    const["b2p"] = _fold_v(b2.astype(f32))
    ts_ = np.linspace(0.0, 1.0, S).astype(f32)
    dt = ts_[1] - ts_[0]
    tm = (np.tanh(ts_[: S - 1, None] * wt[None, :].astype(f32)
                  + bt[None, :].astype(f32)) * dt).astype(f32)   # [S-1, H]
    const["tmdtp"] = np.ascontiguousarray(
        tm.T.reshape(2, P, S - 1).transpose(1, 0, 2))            # [128, 2, S-1]
    const["goutp"] = _fold_v(g_out.astype(f32))
    const["beoutp"] = _fold_v(be_out.astype(f32))
    c1 = float(b1.astype(np.float64).mean())
    has_b3 = bool(np.any(b3))
    if has_b3:
        const["b3row"] = np.ascontiguousarray(b3.astype(f32)[None, :]).astype(bf)
    return const, c1, has_b3


def shard_inputs(x, h, ncores):
    import ml_dtypes
    bf = ml_dtypes.bfloat16
    B = x.shape[0]
    BC = B // ncores
    per_core = []
    for i in range(ncores):
        sl = slice(i * BC, (i + 1) * BC)
        xc = _fold_bm(np.asarray(x[sl], dtype=np.float32))      # [128, 2, BC] f32
        hc = _fold_bm(np.asarray(h[sl], dtype=np.float32))
        per_core.append({
            "xbf": xc.astype(bf),
            "hf": hc,
            "hbf": hc.astype(bf),
        })
    return per_core, BC



def _split_waits_bir(bir_bytes):
    """This container's walrus build rejects >1 embedded sync-wait per
    instruction (and any wait on a Drain). Spill surplus waits into bare
    EventSemaphore instructions just before the owner -- engine program order
    makes the ordering semantics identical, walrus encodes each happily."""
    import json
    m = json.loads(bir_bytes)
    cnt = 0
    for fn in m.get("functions", []):
        for blk in fn.get("blocks", []):
            out = []
            for inst in blk.get("instructions", []):
                body = inst
                si = body.get("sync_info")
                opcode = body.get("opcode")
                waits = (si or {}).get("on_wait") or []
                keep = 0 if opcode == "Drain" else 1
                if si and len(waits) > keep:
                    nspill = len(waits) - keep
                    for w in waits[:nspill]:
                        cnt += 1
                        out.append({
                            "name": f"WSPLIT-{cnt}",
                            "engine": body["engine"],
                            "opcode": "EventSemaphore",
                            "ins": [],
                            "outs": [],
                            "sync_info": {"on_wait": [w], "on_update": []},
                        })
                    si["on_wait"] = waits[nspill:]
                out.append(inst)
            blk["instructions"] = out
    return json.dumps(m).encode()


_BIR_FIX_DONE = False


def _install_bir_fix():
    """Wrap concourse's compile entrypoint so every NEFF build (including the
    one bass2jax triggers under axon) goes through _split_waits_bir."""
    global _BIR_FIX_DONE
    if _BIR_FIX_DONE:
        return
    import sys
    from concourse import bass_utils as bu

    orig = bu.compile_bir_kernel

    def wrapped(bir_json, tmpdir, neff_name="file.neff"):
        if isinstance(bir_json, str):
            bir_json = bir_json.encode()
        return orig(_split_waits_bir(bir_json), tmpdir, neff_name)

    bu.compile_bir_kernel = wrapped
    b2j = sys.modules.get("concourse.bass2jax")
    if b2j is not None and getattr(b2j, "compile_bir_kernel", None) is orig:
        b2j.compile_bir_kernel = wrapped
    _BIR_FIX_DONE = True

_NC_CACHE = {}
TRACE = False          # set by test.py to capture an NTFF profile
LAST_RESULTS = None    # BassKernelResults of the most recent run


def kernel(x, h, W1, b1, g1, be1, W2, b2, W3, b3, wt, bt, g_out, be_out,
           adapt_steps):
    _install_bir_fix()
    from concourse import bass_utils

    S = int(adapt_steps)
    x = np.asarray(x)
    h = np.asarray(h)
    B, H = h.shape
    assert B % NCORES == 0

    f32 = np.float32
    const, c1, has_b3 = prepare(
        np.asarray(W1, f32), np.asarray(b1, f32), np.asarray(g1, f32),
        np.asarray(be1, f32), np.asarray(W2, f32), np.asarray(b2, f32),
        np.asarray(W3, f32), np.asarray(b3, f32), np.asarray(wt, f32),
        np.asarray(bt, f32), np.asarray(g_out, f32), np.asarray(be_out, f32), S)

    shards, BC = shard_inputs(x, h, NCORES)

    has_b1 = bool(np.any(np.asarray(b1)))
    key = (BC, S, has_b3, has_b1)
    if key not in _NC_CACHE:
        shapes = {k: v.shape for k, v in const.items()}
        shapes.update({"xbf": (P, 2, BC), "hf": (P, 2, BC), "hbf": (P, 2, BC)})
        _NC_CACHE[key] = build_nc(BC, S, c1, has_b3, shapes, has_b1=has_b1)
    nc = _NC_CACHE[key]

    in_maps = [{**const, **sh} for sh in shards]
    res = bass_utils.run_bass_kernel_spmd(nc, in_maps,
                                          core_ids=list(range(NCORES)),
                                          trace=TRACE)
    global LAST_RESULTS
    LAST_RESULTS = res
    out = np.empty((B, H), dtype=np.float32)
    for i in range(NCORES):
        oc = res.results[i]["out"]                      # [128, 2, BC]
        out[i * BC:(i + 1) * BC] = oc.transpose(2, 1, 0).reshape(BC, H)
    return out



# revision 3
# speedup vs baseline: 1.6135x; 1.6135x over previous
"""Trainium2 Bass kernel v2: LiquidCell (9-step Euler scan over 3-layer MLP+LN).

Changes vs v1 baseline (7.86 ms):
- fp8e4 DoubleRow matmuls for all three GEMMs (weights host-scaled x16 into
  e4m3 range; the x16 cancels in LN for L1, folds into the gelu scale for L2
  and into tmdt for L3).
- W1 column-centered on host so mean(z1)=0 exactly: no mean matmuls, no
  subtract, var = E[z^2].
- rsqrt of the LN variance: exact ACT Ln/Exp only at step 0 (one table-load
  pair per outer iter), then one DVE Newton iteration per step seeded by the
  previous step's rs (h moves ~0.4%/step, so 1 iteration hits bf16 precision).
- Engine rebalance: L1 PSUM drains on Pool+ACT, GELUs on ACT (L1 in two
  halves, L2 straight from PSUM, both writing fp8), q/u/Newton/casts/h-update
  on DVE.
"""

import numpy as np

P = 128
NCORES = 8
BT = 512          # batch tile (matmul free dim)
G = 4             # tiles interleaved per loop body
LN_EPS = 1e-5
A1 = 16.0         # host scale folded into W1 (cancels in LN)
A2 = 16.0         # host scale folded into W2 (divided out in gelu2 scale)
A3 = 16.0         # host scale folded into W3 (divided out in tmdt)


def _emit(nc, tc, d, BC, S, flags):
    import concourse.mybir as mybir
    from concourse.bass import ds, ts
    from contextlib import ExitStack

    f32 = mybir.dt.float32
    bf16 = mybir.dt.bfloat16
    fp8 = mybir.dt.float8e4
    AF = mybir.ActivationFunctionType
    OP = mybir.AluOpType
    ET = mybir.EngineType
    DR = mybir.MatmulPerfMode.DoubleRow

    has_b1 = flags["has_b1"]
    has_aff1 = flags["has_aff1"]
    has_b2 = flags["has_b2"]
    has_b3 = flags["has_b3"]

    NS = S - 1            # Euler steps
    H4 = 1024
    H2 = 512
    n_m1 = H4 // P             # 8
    n_m2 = H2 // P             # 4
    n_c = 2                    # H/P

    with ExitStack() as ctx:
        singles = ctx.enter_context(tc.tile_pool(name="singles", bufs=1))
        pool_io = ctx.enter_context(tc.tile_pool(name="io", bufs=G))
        pool_big = ctx.enter_context(tc.tile_pool(name="big", bufs=G))
        pool_f32 = ctx.enter_context(tc.tile_pool(name="f32tmp", bufs=6))
        pool_rf = ctx.enter_context(tc.tile_pool(name="rowf", bufs=6))
        pool_rn = ctx.enter_context(tc.tile_pool(name="rown", bufs=2))
        psum_mm = ctx.enter_context(tc.tile_pool(name="psmm", bufs=6, space="PSUM"))
        psum_st = ctx.enter_context(tc.tile_pool(name="psst", bufs=2, space="PSUM"))

        def load(name, dtype):
            t = singles.tile(list(d[name].shape), dtype, tag=name)
            nc.sync.dma_start(out=t, in_=d[name][:])
            return t

        w1t8 = load("w1t8", fp8)      # [128, 2, 2, 1024]
        w2t8 = load("w2t8", fp8)      # [128, 4, 2, 512]
        w3t8 = load("w3t8", fp8)      # [128, 2, 2, 256]
        tmdtp = load("tmdtp", f32)    # [128, 2, NS]
        goutp = load("goutp", f32)
        beoutp = load("beoutp", f32)
        b1cp = load("b1cp", f32) if has_b1 else None
        g1p = load("g1p", f32) if has_aff1 else None
        be1p = load("be1p", f32) if has_aff1 else None
        b2p = load("b2p", f32) if has_b2 else None
        b3row = load("b3row", bf16) if has_b3 else None

        ones_ss = singles.tile([P, 1], bf16)
        nc.vector.memset(ones_ss, 1.0 / H4)
        ones1_bf = singles.tile([1, P], bf16)
        nc.vector.memset(ones1_bf, 1.0)
        ones_hf = singles.tile([P, 1], f32)
        nc.vector.memset(ones_hf, 1.0 / (n_c * P))
        ones1_f = singles.tile([1, P], f32)
        nc.vector.memset(ones1_f, 1.0)
        eps_t = singles.tile([1, 1], f32)
        nc.vector.memset(eps_t, LN_EPS * A1 * A1)
        eps_o = singles.tile([1, 1], f32)
        nc.vector.memset(eps_o, LN_EPS)
        if has_b3:
            onesrow_bf = singles.tile([1, BT], bf16)
            nc.vector.memset(onesrow_bf, 1.0)

        # persistent row buffers (partition 0): per-tile slices of [1, G*BT]
        rs_all = singles.tile([1, G * BT], bf16, tag="rs_all")
        var_all = singles.tile([1, G * BT], bf16, tag="var_all")
        lnv_all = singles.tile([1, G * BT], bf16, tag="lnv_all")

        class T:
            pass

        def make_tile(off, j):
            t = T()
            t.off = off
            t.j = j
            t.x8 = pool_io.tile([P, 2, BT], fp8, tag="x8")
            t.hT = pool_io.tile([P, 2, BT], f32, tag="hT")
            t.h8 = pool_io.tile([P, 2, BT], fp8, tag="h8")
            t.s = pool_big.tile([P, n_m1, BT], bf16, tag="s")     # z1c
            t.q = pool_big.tile([P, n_m1, BT], bf16, tag="q")     # squares / u
            t.z1g = pool_big.tile([P, n_m1, BT], fp8, tag="z1g")
            t.z2 = pool_big.tile([P, n_m2, BT], fp8, tag="z2")
            t.rsB = pool_big.tile([P, BT], bf16, tag="rsB")
            nc.sync.dma_start(out=t.x8, in_=d["x8"][:, :, ds(off, BT)])
            nc.sync.dma_start(out=t.hT, in_=d["hf"][:, :, ds(off, BT)])
            nc.sync.dma_start(out=t.h8, in_=d["h8"][:, :, ds(off, BT)])
            return t

        # ---------------- phase A: layer-1 matmuls + drains + squares -------
        def pA(t):
            t.ss_ps = psum_st.tile([1, BT], f32, tag="st")
            for m in range(n_m1):
                zps = psum_mm.tile([P, BT], f32, tag="mm")
                nc.tensor.matmul(zps, lhsT=w1t8[:, 0, :, ts(m, P)],
                                 rhs=t.h8, start=True, stop=False, perf_mode=DR)
                nc.tensor.matmul(zps, lhsT=w1t8[:, 1, :, ts(m, P)],
                                 rhs=t.x8, start=False, stop=True, perf_mode=DR)
                # drain PSUM -> SBUF bf16 (z1c); gpsimd can't touch PSUM,
                # so split the 8 drains between ACT and DVE
                if has_b1:
                    nc.scalar.activation(out=t.s[:, m, :], in_=zps,
                                         func=AF.Identity,
                                         bias=b1cp[:, m:m + 1], scale=1.0)
                elif m % 2 == 0:
                    nc.scalar.copy(out=t.s[:, m, :], in_=zps)
                else:
                    nc.vector.tensor_copy(out=t.s[:, m, :], in_=zps)
            nc.vector.tensor_mul(out=t.q, in0=t.s, in1=t.s)

        # ---------------- phase B: ss matmuls (+ step-0 var copy) ----------
        def pB_ss(t):
            for m in range(n_m1):
                nc.tensor.matmul(t.ss_ps, lhsT=ones_ss, rhs=t.q[:, m, :],
                                 start=(m == 0), stop=(m == n_m1 - 1))

        def pB_var_copy(t):
            # step 0: var -> var_all slice (for exact ln/exp rsqrt)
            nc.vector.tensor_copy(out=var_all[:, ds(t.j * BT, BT)], in_=t.ss_ps)

        def rs_exact_all():
            # rs_all = exp(-0.5*ln(var+eps))  for all G tiles at once
            nc.scalar.activation(out=lnv_all, in_=var_all, func=AF.Ln,
                                 bias=eps_t[0:1, 0:1], scale=1.0)
            nc.scalar.activation(out=rs_all, in_=lnv_all, func=AF.Exp,
                                 bias=0.0, scale=-0.5)

        def pB_newton(t):
            # one Newton step: rs' = rs*(1.5 - 0.5*var*rs^2), in rows
            sl = (slice(0, 1), ds(t.j * BT, BT))
            t2 = pool_rn.tile([1, BT], bf16, tag="rn")
            nc.vector.tensor_mul(out=t2, in0=rs_all[sl], in1=rs_all[sl])
            pr = pool_rn.tile([1, BT], f32, tag="rp")
            nc.vector.tensor_mul(out=pr, in0=t2, in1=t.ss_ps)
            w = pool_rn.tile([1, BT], bf16, tag="rw")
            nc.vector.tensor_scalar(out=w, in0=pr, scalar1=-0.5, scalar2=1.5,
                                    op0=OP.mult, op1=OP.add)
            nc.vector.tensor_mul(out=rs_all[sl], in0=w, in1=rs_all[sl])

        # ---------------- phase C: broadcast rs, normalize, gelu -----------
        def pC(t):
            bc = psum_mm.tile([P, BT], f32, tag="mm")
            nc.tensor.matmul(bc, lhsT=ones1_bf,
                             rhs=rs_all[0:1, ds(t.j * BT, BT)],
                             start=True, stop=True)
            nc.scalar.copy(out=t.rsB, in_=bc)
            u = t.q  # reuse squares tile
            for h in range(2):
                hs = slice(4 * h, 4 * h + 4)
                nc.vector.tensor_mul(
                    out=u[:, hs, :], in0=t.s[:, hs, :],
                    in1=t.rsB[:, None, :].to_broadcast((P, 4, BT)))
                if has_aff1:
                    for m in range(4 * h, 4 * h + 4):
                        nc.scalar.activation(out=t.z1g[:, m, :],
                                             in_=u[:, m, :], func=AF.Gelu,
                                             bias=be1p[:, m:m + 1],
                                             scale=g1p[:, m:m + 1])
                else:
                    nc.scalar.activation(out=t.z1g[:, hs, :], in_=u[:, hs, :],
                                         func=AF.Gelu, bias=0.0, scale=1.0)

        # ---------------- phase D: layer 2 ---------------------------------
        def pD(t):
            for m in range(n_m2):
                zps = psum_mm.tile([P, BT], f32, tag="mm")
                for p8 in range(4):
                    nc.tensor.matmul(zps, lhsT=w2t8[:, p8, :, ts(m, P)],
                                     rhs=t.z1g[:, 2 * p8:2 * p8 + 2, :],
                                     start=(p8 == 0), stop=(p8 == 3),
                                     perf_mode=DR)
                nc.scalar.activation(out=t.z2[:, m, :], in_=zps, func=AF.Gelu,
                                     bias=(b2p[:, m:m + 1] if has_b2 else 0.0),
                                     scale=1.0 / A2)

        # ---------------- phase E: layer 3 + h update ----------------------
        def pE(t, s):
            for c in range(n_c):
                dps = psum_mm.tile([P, BT], f32, tag="mm")
                for p4 in range(2):
                    nc.tensor.matmul(dps, lhsT=w3t8[:, p4, :, ts(c, P)],
                                     rhs=t.z2[:, 2 * p4:2 * p4 + 2, :],
                                     start=(p4 == 0),
                                     stop=(p4 == 1) and not has_b3,
                                     perf_mode=DR)
                if has_b3:
                    nc.tensor.matmul(dps, lhsT=b3row[0:1, ts(c, P)],
                                     rhs=onesrow_bf, start=False, stop=True)
                # h += tmdt * dh   (tmdt has dt/A3 folded in)
                nc.vector.scalar_tensor_tensor(
                    out=t.hT[:, c, :], in0=dps, scalar=tmdtp[:, c, s:s + 1],
                    in1=t.hT[:, c, :], op0=OP.mult, op1=OP.add)
            if s < NS - 1:
                nc.gpsimd.tensor_copy(out=t.h8, in_=t.hT)

        # ---------------- final LayerNorm + store --------------------------
        def pfinal(t):
            mu_ps = psum_st.tile([1, BT], f32, tag="st")
            for c in range(n_c):
                nc.tensor.matmul(mu_ps, lhsT=ones_hf, rhs=t.hT[:, c, :],
                                 start=(c == 0), stop=(c == n_c - 1))
            q2a = pool_f32.tile([P, BT], f32, tag="f32")
            nc.vector.tensor_mul(out=q2a, in0=t.hT[:, 0, :], in1=t.hT[:, 0, :])
            q2b = pool_f32.tile([P, BT], f32, tag="f32")
            nc.vector.tensor_mul(out=q2b, in0=t.hT[:, 1, :], in1=t.hT[:, 1, :])
            tsum = pool_f32.tile([P, BT], f32, tag="f32")
            nc.vector.tensor_add(out=tsum, in0=q2a, in1=q2b)
            ss_ps = psum_st.tile([1, BT], f32, tag="st")
            nc.tensor.matmul(ss_ps, lhsT=ones_hf, rhs=tsum, start=True, stop=True)
            mu2 = pool_rf.tile([1, BT], f32, tag="rf")
            nc.vector.tensor_copy(out=mu2, in_=mu_ps)
            msq2 = pool_rf.tile([1, BT], f32, tag="rf")
            nc.vector.tensor_mul(out=msq2, in0=mu2, in1=mu2)
            var2 = pool_rf.tile([1, BT], f32, tag="rf")
            nc.vector.tensor_sub(out=var2, in0=ss_ps, in1=msq2)
            lnv2 = pool_rf.tile([1, BT], f32, tag="rf")
            nc.scalar.activation(out=lnv2, in_=var2, func=AF.Ln,
                                 bias=eps_o[0:1, 0:1], scale=1.0)
            rs2 = pool_rf.tile([1, BT], f32, tag="rf")
            nc.scalar.activation(out=rs2, in_=lnv2, func=AF.Exp,
                                 bias=0.0, scale=-0.5)
            bmu = psum_mm.tile([P, BT], f32, tag="mm")
            nc.tensor.matmul(bmu, lhsT=ones1_f, rhs=mu2, start=True, stop=True)
            brs = psum_mm.tile([P, BT], f32, tag="mm")
            nc.tensor.matmul(brs, lhsT=ones1_f, rhs=rs2, start=True, stop=True)
            for c in range(n_c):
                dd = pool_f32.tile([P, BT], f32, tag="f32")
                nc.vector.tensor_sub(out=dd, in0=t.hT[:, c, :], in1=bmu)
                nc.vector.tensor_mul(out=dd, in0=dd, in1=brs)
                oc = pool_f32.tile([P, BT], f32, tag="f32")
                nc.scalar.activation(out=oc, in_=dd, func=AF.Identity,
                                     bias=beoutp[:, c:c + 1],
                                     scale=goutp[:, c:c + 1])
                nc.sync.dma_start(out=d["out"][:, c, ds(t.off, BT)], in_=oc)

        assert BC % (G * BT) == 0
        with tc.For_i(0, BC, G * BT,
                      hint_engines=(ET.PE, ET.Activation, ET.DVE)) as i0:
            tiles = []
            for g in range(G):
                tiles.append(make_tile(i0 if g == 0 else i0 + g * BT, g))
            for s in range(NS):
                if s == 0:
                    for j in range(G):
                        pA(tiles[j])
                        if j >= 1:
                            pB_ss(tiles[j - 1])
                            pB_var_copy(tiles[j - 1])
                    pB_ss(tiles[G - 1])
                    pB_var_copy(tiles[G - 1])
                    rs_exact_all()
                    for j in range(G):
                        pC(tiles[j])
                else:
                    for j in range(G):
                        pA(tiles[j])
                        if j >= 1:
                            pB_ss(tiles[j - 1])
                            pB_newton(tiles[j - 1])
                        if j >= 2:
                            pC(tiles[j - 2])
                    pB_ss(tiles[G - 1])
                    pB_newton(tiles[G - 1])
                    pC(tiles[G - 2])
                    pC(tiles[G - 1])
                for j in range(G):
                    pD(tiles[j])
                    pE(tiles[j], s)
            for j in range(G):
                pfinal(tiles[j])


def build_nc(BC, S, flags, shapes):
    import concourse.bass as bass
    import concourse.mybir as mybir
    import concourse.tile as tile

    f32 = mybir.dt.float32
    bf16 = mybir.dt.bfloat16
    fp8 = mybir.dt.float8e4
    nc = bass.Bass("TRN2", target_bir_lowering=False, debug=False)
    d = {}
    dts = {"x8": fp8, "hf": f32, "h8": fp8, "w1t8": fp8, "w2t8": fp8,
           "w3t8": fp8, "tmdtp": f32, "goutp": f32, "beoutp": f32,
           "b1cp": f32, "g1p": f32, "be1p": f32, "b2p": f32, "b3row": bf16}
    for name, shape in shapes.items():
        d[name] = nc.dram_tensor(name, list(shape), dts[name],
                                 kind="ExternalInput")[:]
    d["out"] = nc.dram_tensor("out", [P, 2, BC], f32, kind="ExternalOutput")[:]
    with tile.TileContext(nc) as tc:
        _emit(nc, tc, d, BC, S, flags)
    return nc


def _fold_w_dr(W):
    """W [M, K] -> DoubleRow lhsT tiles [128, K/256, 2, M]:
    [kp, pr, i, j] = W[j, (2*pr+i)*128 + kp]."""
    M, K = W.shape
    return np.ascontiguousarray(
        W.T.reshape(K // 256, 2, P, M).transpose(2, 0, 1, 3))


def _fold_v(v):
    """v [F] -> [128, F/128] with [p, c] = v[c*128+p]."""
    return np.ascontiguousarray(v.reshape(-1, P).T)


def _fold_bm(a):
    """a [N, F] batch-major -> feature-major [128, F/128, N]."""
    n, f = a.shape
    return np.ascontiguousarray(a.T.reshape(f // P, P, n).transpose(1, 0, 2))


def _to_fp8(a):
    import ml_dtypes
    return np.clip(a, -240.0, 240.0).astype(ml_dtypes.float8_e4m3)


def prepare(W1, b1, g1, be1, W2, b2, W3, b3, wt, bt, g_out, be_out, S):
    f32 = np.float32
    const = {}
    W1c = (W1 - W1.mean(axis=0, keepdims=True)) * A1
    const["w1t8"] = _to_fp8(_fold_w_dr(W1c))
    const["w2t8"] = _to_fp8(_fold_w_dr(W2 * A2))
    const["w3t8"] = _to_fp8(_fold_w_dr(W3 * A3))
    ts_ = np.linspace(0.0, 1.0, S).astype(f32)
    dt = ts_[1] - ts_[0]
    tm = (np.tanh(ts_[: S - 1, None] * wt[None, :].astype(f32)
                  + bt[None, :].astype(f32)) * (dt / A3)).astype(f32)
    const["tmdtp"] = np.ascontiguousarray(
        tm.T.reshape(2, P, S - 1).transpose(1, 0, 2))        # [128, 2, S-1]
    const["goutp"] = _fold_v(g_out.astype(f32))
    const["beoutp"] = _fold_v(be_out.astype(f32))
    flags = {
        "has_b1": bool(np.any(b1)),
        "has_aff1": bool(np.any(be1)) or bool(np.any(g1 != 1.0)),
        "has_b2": bool(np.any(b2)),
        "has_b3": bool(np.any(b3)),
    }
    if flags["has_b1"]:
        b1c = (b1 - b1.mean()) * A1
        const["b1cp"] = _fold_v(b1c.astype(f32))
    if flags["has_aff1"]:
        const["g1p"] = _fold_v(g1.astype(f32))
        const["be1p"] = _fold_v(be1.astype(f32))
    if flags["has_b2"]:
        const["b2p"] = _fold_v(b2.astype(f32))
    if flags["has_b3"]:
        import ml_dtypes
        const["b3row"] = np.ascontiguousarray(
            (b3 * (1.0)).astype(f32)[None, :]).astype(ml_dtypes.bfloat16)
    return const, flags


def shard_inputs(x, h, ncores):
    B = x.shape[0]
    BC = B // ncores
    per_core = []
    for i in range(ncores):
        sl = slice(i * BC, (i + 1) * BC)
        xc = _fold_bm(np.asarray(x[sl], dtype=np.float32))
        hc = _fold_bm(np.asarray(h[sl], dtype=np.float32))
        per_core.append({
            "x8": _to_fp8(xc),
            "hf": hc,
            "h8": _to_fp8(hc),
        })
    return per_core, BC


def _split_waits_bir(bir_bytes):
    """This container's walrus build rejects >1 embedded sync-wait per
    instruction (and any wait on a Drain). Spill surplus waits into bare
    EventSemaphore instructions just before the owner -- engine program order
    makes the ordering semantics identical, walrus encodes each happily."""
    import json
    m = json.loads(bir_bytes)
    cnt = 0
    for fn in m.get("functions", []):
        for blk in fn.get("blocks", []):
            out = []
            for inst in blk.get("instructions", []):
                body = inst
                si = body.get("sync_info")
                opcode = body.get("opcode")
                waits = (si or {}).get("on_wait") or []
                keep = 0 if opcode == "Drain" else 1
                if si and len(waits) > keep:
                    nspill = len(waits) - keep
                    for w in waits[:nspill]:
                        cnt += 1
                        out.append({
                            "name": f"WSPLIT-{cnt}",
                            "engine": body["engine"],
                            "opcode": "EventSemaphore",
                            "ins": [],
                            "outs": [],
                            "sync_info": {"on_wait": [w], "on_update": []},
                        })
                    si["on_wait"] = waits[nspill:]
                out.append(inst)
            blk["instructions"] = out
    return json.dumps(m).encode()


_BIR_FIX_DONE = False


def _install_bir_fix():
    global _BIR_FIX_DONE
    if _BIR_FIX_DONE:
        return
    import sys
    from concourse import bass_utils as bu

    orig = bu.compile_bir_kernel

    def wrapped(bir_json, tmpdir, neff_name="file.neff"):
        if isinstance(bir_json, str):
            bir_json = bir_json.encode()
        return orig(_split_waits_bir(bir_json), tmpdir, neff_name)

    bu.compile_bir_kernel = wrapped
    b2j = sys.modules.get("concourse.bass2jax")
    if b2j is not None and getattr(b2j, "compile_bir_kernel", None) is orig:
        b2j.compile_bir_kernel = wrapped
    _BIR_FIX_DONE = True


_NC_CACHE = {}
TRACE = False          # set by test.py to capture an NTFF profile
LAST_RESULTS = None    # BassKernelResults of the most recent run


def kernel(x, h, W1, b1, g1, be1, W2, b2, W3, b3, wt, bt, g_out, be_out,
           adapt_steps):
    _install_bir_fix()
    from concourse import bass_utils

    S = int(adapt_steps)
    x = np.asarray(x)
    h = np.asarray(h)
    B, H = h.shape
    assert B % NCORES == 0

    f32 = np.float32
    const, flags = prepare(
        np.asarray(W1, f32), np.asarray(b1, f32), np.asarray(g1, f32),
        np.asarray(be1, f32), np.asarray(W2, f32), np.asarray(b2, f32),
        np.asarray(W3, f32), np.asarray(b3, f32), np.asarray(wt, f32),
        np.asarray(bt, f32), np.asarray(g_out, f32), np.asarray(be_out, f32),
        S)

    shards, BC = shard_inputs(x, h, NCORES)

    key = (BC, S, tuple(sorted(flags.items())))
    if key not in _NC_CACHE:
        shapes = {k: v.shape for k, v in const.items()}
        shapes.update({"x8": (P, 2, BC), "hf": (P, 2, BC), "h8": (P, 2, BC)})
        _NC_CACHE[key] = build_nc(BC, S, flags, shapes)
    nc = _NC_CACHE[key]

    in_maps = [{**const, **sh} for sh in shards]
    res = bass_utils.run_bass_kernel_spmd(nc, in_maps,
                                          core_ids=list(range(NCORES)),
                                          trace=TRACE)
    global LAST_RESULTS
    LAST_RESULTS = res
    out = np.empty((B, H), dtype=np.float32)
    for i in range(NCORES):
        oc = res.results[i]["out"]                      # [128, 2, BC]
        out[i * BC:(i + 1) * BC] = oc.transpose(2, 1, 0).reshape(BC, H)
    return out
